# revision 54
# baseline (speedup 1.0000x reference)
"""Trainium2 Bass kernel for windowed causal multi-head attention.

Problem (hardcoded): x [4, 2048, 1024], 16 heads x 64 dim, rotary embedding,
causal attention with left window 256, fused QKV/out projections.

Sharding: 8 cores = (batch b in 0..3) x (head-group g in 0..1). Each core
computes batch b, heads [g*8, (g+1)*8) and produces a partial output
[2048, 1024] (its head-group's contribution to the out-projection). The host
sums the two partials per batch and adds the output bias (which also absorbs
bv: softmax weights sum to 1, so v+bv contributes bv@Wo to every output row).

Device-side layout strategy (transpose-free, all matmuls bf16):
  - Projections compute qT/kT [hd, seq] with head-dim on partitions
    (lhsT = W chunk) and v [seq, hd] naturally (lhsT = xT chunk). Rotary
    rotate_half is a 128x128 constant matmul plus two elementwise multiplies
    against host-precomputed cos/sin rows.
  - Per (256-query block, head pair, side): scores S^T [keys, queries] are
    computed as 4 matmuls over the 4 banded key blocks; the two middle key
    blocks are shared by both 128-query halves and run at N=256. Layout is
    a [128, 6, 128] PSUM tile: plane m = 2*jb_rel + query_half, so the
    half-planes needed for masks/presum/PV fall on regular strides.
  - Softmax: one exp per side (no max subtraction: |scores| < ~10), band
    masks as two {0,1}-multiplies on strided plane pairs (DVE), then the
    3 key-block planes are pre-summed on GpSimd so each side's softmax
    denominator is a single ones-matmul at N=256 (PE cost 1/3 of per-block
    denominator matmuls). Context C^T accumulates the banded key blocks with
    the same N=256 middle-block sharing; normalized with a reciprocal
    multiply.
  - Out projection consumes C^T directly as lhsT (K = head dims).
  - Stage A (K/V projections) and stage B (Q + attention + out-proj) are
    interleaved per 256-position block so Scalar/DVE/GpSimd load spreads.
  - Engines execute queues in order, so emission order is the schedule:
    attention tails lag their scores by DEPTH iterations so the PE never
    waits on the exp->mask->presum chain.
"""

import numpy as np

import concourse.bass as bass
import concourse.mybir as mybir
import concourse.tile as tile
from concourse import bacc
from concourse import bass_utils

B, S, E = 4, 2048, 1024
H, D = 16, 64
W = 512          # per-core head-group width (8 heads x 64)
QB = 256         # query block
NQB = S // QB    # 8
NKC = E // 128   # 8 contraction chunks for projections
PAIRS = 4        # head pairs per core (128 cols each)
SCALE = 1.0 / 8.0  # 1/sqrt(D)
DEPTH = 4        # attention-tail pipeline depth (in (qb,c) iterations)

F32 = mybir.dt.float32
MDT = mybir.dt.bfloat16

_STATE = None


def _build():
    nc = bacc.Bacc("TRN2", target_bir_lowering=False, debug=False, num_devices=8)

    xtc = nc.dram_tensor("xtc", [NQB, 128, NKC, QB], MDT,
                         kind="ExternalInput").ap()
    wq = nc.dram_tensor("wq", [PAIRS, 128, NKC, 128], MDT,
                        kind="ExternalInput").ap()
    wk = nc.dram_tensor("wk", [PAIRS, 128, NKC, 128], MDT,
                        kind="ExternalInput").ap()
    wv = nc.dram_tensor("wv", [128, NKC, W], MDT, kind="ExternalInput").ap()
    wo = nc.dram_tensor("wo", [128, PAIRS, E], MDT, kind="ExternalInput").ap()
    bqc = nc.dram_tensor("bqc", [128, PAIRS], F32, kind="ExternalInput").ap()
    onesd = nc.dram_tensor("onesd", [128, 64], MDT, kind="ExternalInput").ap()
    rt = nc.dram_tensor("rt", [128, 128], MDT, kind="ExternalInput").ap()
    cosh = nc.dram_tensor("cosh", [64, S], MDT, kind="ExternalInput").ap()
    sinh = nc.dram_tensor("sinh", [64, S], MDT, kind="ExternalInput").ap()
    mask4 = nc.dram_tensor("mask4", [128, 4, 128], MDT,
                           kind="ExternalInput").ap()
    out = nc.dram_tensor("out", [S, E], F32, kind="ExternalOutput").ap()

    with tile.TileContext(nc) as tc:
        with tc.tile_pool(name="res", bufs=1) as res, \
             tc.tile_pool(name="work", bufs=2) as work, \
             tc.tile_pool(name="attn", bufs=2) as attn, \
             tc.tile_pool(name="ps", bufs=4, space="PSUM") as ps, \
             tc.tile_pool(name="ps_s", bufs=2, space="PSUM") as ps_s:

            # --- resident constants / weights ---
            xt_sb = res.tile([128, NKC, S], MDT)     # resident x^T
            cos2_sb = res.tile([128, S], MDT)
            sin2_sb = res.tile([128, S], MDT)
            wq_sb = res.tile([128, NKC, W], MDT)
            wk_sb = res.tile([128, NKC, W], MDT)
            wv_sb = res.tile([128, NKC, W], MDT)
            wo_sb = res.tile([128, PAIRS, E], MDT)
            bqc_sb = res.tile([128, PAIRS], F32)
            onesd_sb = res.tile([128, 64], MDT)
            rt_sb = res.tile([128, 128], MDT)
            mask4_sb = res.tile([128, 4, 128], MDT)
            kT_sb = res.tile([128, PAIRS, S], MDT)   # rotated K^T per pair
            v_sb = res.tile([128, S // 128, W], MDT)  # V (seq-major tiles)

            # Startup DMAs: HBM pull runs at a fixed ~260GB/s regardless of
            # queue count, so a single sync queue in strict consumption
            # order is the best schedule. Early bytes are minimized: cos/sin
            # are loaded once for positions 0:1024 (blocks 0-3) and
            # duplicated to the upper partition half SBUF->SBUF on scalar;
            # the 1024:2048 halves ride at the end of the chain.
            # first halves split so pair0's first matmuls can start ~3us
            # earlier while the rest streams in
            nc.sync.dma_start(out=wk_sb[:, 0:4, 0:128], in_=wk[0][:, 0:4, :])
            nc.sync.dma_start(out=xt_sb[:, 0:4, 0:QB], in_=xtc[0][:, 0:4, :])
            nc.sync.dma_start(out=wk_sb[:, 4:8, 0:128], in_=wk[0][:, 4:8, :])
            nc.sync.dma_start(out=xt_sb[:, 4:8, 0:QB], in_=xtc[0][:, 4:8, :])
            nc.sync.dma_start(out=wk_sb[:, :, 128:256], in_=wk[1])
            nc.sync.dma_start(out=wk_sb[:, :, 256:384], in_=wk[2])
            nc.sync.dma_start(out=wk_sb[:, :, 384:512], in_=wk[3])
            nc.sync.dma_start(out=rt_sb, in_=rt)
            nc.sync.dma_start(out=cos2_sb[0:64, 0:1024], in_=cosh[:, 0:1024])
            nc.sync.dma_start(out=sin2_sb[0:64, 0:1024], in_=sinh[:, 0:1024])
            # dups stay on the same queue as their source loads: DMA->DMA
            # ordering is only guaranteed by queue FIFO
            nc.sync.dma_start(out=cos2_sb[64:128, 0:1024],
                              in_=cos2_sb[0:64, 0:1024])
            nc.sync.dma_start(out=sin2_sb[64:128, 0:1024],
                              in_=sin2_sb[0:64, 0:1024])
            nc.sync.dma_start(out=xt_sb[:, :, QB:2 * QB], in_=xtc[1])
            nc.sync.dma_start(out=xt_sb[:, :, 2 * QB:3 * QB], in_=xtc[2])
            nc.sync.dma_start(out=bqc_sb, in_=bqc)
            for c in range(PAIRS):
                nc.sync.dma_start(out=wq_sb[:, :, c * 128:(c + 1) * 128],
                                  in_=wq[c])
            nc.sync.dma_start(out=xt_sb[:, :, 3 * QB:4 * QB], in_=xtc[3])
            nc.sync.dma_start(out=wv_sb, in_=wv)
            nc.sync.dma_start(out=wo_sb, in_=wo)
            nc.sync.dma_start(out=cos2_sb[0:64, 1024:2048],
                              in_=cosh[:, 1024:2048])
            nc.sync.dma_start(out=sin2_sb[0:64, 1024:2048],
                              in_=sinh[:, 1024:2048])
            nc.sync.dma_start(out=cos2_sb[64:128, 1024:2048],
                              in_=cos2_sb[0:64, 1024:2048])
            nc.sync.dma_start(out=sin2_sb[64:128, 1024:2048],
                              in_=sin2_sb[0:64, 1024:2048])
            # scalar queue: free until its first kraw copy (~13us);
            # DMA->compute deps are tracked across queues, so these can
            # ride here without diluting the sync chain's priority order
            nc.scalar.dma_start(out=mask4_sb, in_=mask4)
            nc.scalar.dma_start(out=onesd_sb, in_=onesd)

            # ---------- Stage A: K^T (rotated) + V for one 256-seq block ----
            a_pend = []
            a_pend_c = []

            def emit_ktail():
                kraw, sl_ = a_pend.pop(0)
                ps_rh = ps.tile([128, QB], F32, tag="dc", bufs=2,
                                name=f"ps_rh_a_{sl_.start}")
                nc.tensor.matmul(ps_rh, rt_sb, kraw, start=True, stop=True)
                rhs_sin = work.tile([128, QB], MDT, tag="rhsin", bufs=3,
                                    name=f"rhs_sin_a_{sl_.start}")
                nc.vector.tensor_mul(out=rhs_sin, in0=ps_rh,
                                     in1=sin2_sb[:, sl_])
                nc.vector.tensor_mul(out=kraw, in0=kraw,
                                     in1=cos2_sb[:, sl_])
                nc.vector.tensor_add(
                    out=kT_sb[:, a_pend_c.pop(0), sl_], in0=kraw,
                    in1=rhs_sin)

            def emit_v(n8):
                sl = slice(n8 * QB, (n8 + 1) * QB)
                xa = xt_sb[:, :, sl]
                for sub in range(2):
                    jb = n8 * 2 + sub
                    ps_v = ps.tile([128, W], F32, tag="pj", bufs=2,
                                   name=f"ps_v_{jb}")
                    for kc in range(NKC):
                        nc.tensor.matmul(
                            ps_v,
                            xa[:, kc, sub * 128:(sub + 1) * 128],
                            wv_sb[:, kc, :],
                            start=(kc == 0), stop=(kc == NKC - 1),
                        )
                    # V copy on DVE: on Scalar it queues ahead of the
                    # exps whose latency gates the sAB score-PSUM ring
                    nc.vector.tensor_copy(out=v_sb[:, jb, :], in_=ps_v)

            def emit_k_pair(n8, c):
                # one K-projection unit: block n8, head pair c
                sl = slice(n8 * QB, (n8 + 1) * QB)
                xa = xt_sb[:, :, sl]
                if c == 0 and 4 <= n8 + 2 <= 7:
                    pre = n8 + 2
                    nc.sync.dma_start(
                        out=xt_sb[:, :, pre * QB:(pre + 1) * QB],
                        in_=xtc[pre])
                ps_k = ps.tile([128, QB], F32, tag="pj", bufs=2,
                               name=f"ps_k_{n8}_{c}")
                for kc in range(NKC):
                    nc.tensor.matmul(
                        ps_k,
                        wk_sb[:, kc, c * 128:(c + 1) * 128],
                        xa[:, kc, :],
                        start=(kc == 0), stop=(kc == NKC - 1),
                    )
                kraw = work.tile([128, QB], MDT, tag="kraw", bufs=4,
                                 name=f"kraw_{n8}_{c}")
                nc.scalar.activation(
                    out=kraw, in_=ps_k,
                    func=mybir.ActivationFunctionType.Copy)
                a_pend.append((kraw, sl))
                a_pend_c.append(c)
                if len(a_pend) > 1:
                    emit_ktail()

            # ---------- Stage B: Q + attention + out-projection -------------
            pending = []          # (qb, c, pABs, presums, cn)
            qb_tails_left = {}
            cn_by_qb = {}
            qrots_by_qb = {}

            def emit_wo(qb):
                cts = cn_by_qb.pop(qb)
                last = qb == NQB - 1
                for sub in range(2):
                    o_sb = work.tile([128, 1024], F32, tag="o_sb",
                                     name=f"o_sb_{qb}_{sub}")
                    rows = slice(qb * QB + sub * 128,
                                 qb * QB + (sub + 1) * 128)
                    for ncol in range(2):
                        # sub1 groups borrow the dc ring so the next
                        # A-burst's first K matmuls (pj ring) don't wait
                        # out-copy latency at the block seam
                        ps_o = ps.tile([128, 512], F32,
                                       tag="pj" if sub == 0 else "dc",
                                       bufs=2,
                                       name=f"ps_o_{qb}_{sub}_{ncol}")
                        for cc in range(PAIRS):
                            nc.tensor.matmul(
                                ps_o,
                                cts[cc][:, sub * 128:(sub + 1) * 128],
                                wo_sb[:, cc, ncol * 512:(ncol + 1) * 512],
                                start=(cc == 0), stop=(cc == PAIRS - 1))
                        osl = slice(ncol * 512, (ncol + 1) * 512)
                        # split copies across Scalar/DVE so exps never queue
                        # behind a burst of out-copies
                        if last:
                            # queues are drained here: halve the copy wall
                            # by splitting across Scalar+DVE
                            nc.scalar.activation(
                                out=o_sb[:, ncol * 512:ncol * 512 + 256],
                                in_=ps_o[:, 0:256],
                                func=mybir.ActivationFunctionType.Copy)
                            nc.vector.tensor_copy(
                                out=o_sb[:, ncol * 512 + 256:(ncol + 1) * 512],
                                in_=ps_o[:, 256:512])
                        elif ncol == 1:
                            nc.vector.tensor_copy(out=o_sb[:, osl],
                                                  in_=ps_o)
                        else:
                            nc.scalar.activation(
                                out=o_sb[:, osl], in_=ps_o,
                                func=mybir.ActivationFunctionType.Copy)
                        if last:
                            eng = (nc.sync, nc.scalar, nc.sync,
                                   nc.gpsimd)[2 * sub + ncol]
                            eng.dma_start(out=out[rows, osl],
                                          in_=o_sb[:, osl])
                    if not last:
                        nc.sync.dma_start(out=out[rows, :], in_=o_sb)

            def emit_tail():
                qb, c, pABs, presums, cn = pending.pop(0)
                ps_dcv = ps.tile([128, 2, QB], F32, tag="dc", bufs=2,
                                 name=f"ps_dcv_{qb}_{c}")
                # Denominators: ones-matmul over the presummed planes.
                # Side A -> partitions 0:64, side B -> 64:128 (tile_position).
                # All den/PV matmuls are emitted A/B-interleaved: the sides
                # occupy disjoint 64-col groups, so adjacent MMs overlap.
                def tp_hs(side):
                    return ((None, (0, 64))[side],
                            slice(64 * side, 64 * side + 64))
                if qb == 0:
                    for mmi in range(2):
                        for side in range(2):
                            tp, hs = tp_hs(side)
                            if mmi == 0:
                                # h0 den = masked diag plane only
                                nc.tensor.matmul(
                                    ps_dcv[hs, 0, 0:128], onesd_sb,
                                    pABs[side][:, 4, :], start=True,
                                    stop=True, tile_position=tp)
                            else:
                                nc.tensor.matmul(
                                    ps_dcv[hs, 0, 128:256], onesd_sb,
                                    presums[side][:, 1, :], start=True,
                                    stop=True, tile_position=tp)
                elif qb == NQB - 1:
                    # Last block: no presum (GpSimd chain would sit on
                    # the drain critical path) - accumulate planes on PE.
                    for half in range(2):
                        csl = slice(half * 128, (half + 1) * 128)
                        for i, m in enumerate((half, half + 2, half + 4)):
                            for side in range(2):
                                tp, hs = tp_hs(side)
                                nc.tensor.matmul(
                                    ps_dcv[hs, 0, csl], onesd_sb,
                                    pABs[side][:, m, :], start=(i == 0),
                                    stop=(i == 2), tile_position=tp)
                else:
                    for side in range(2):
                        tp, hs = tp_hs(side)
                        nc.tensor.matmul(
                            ps_dcv[hs, 0, :], onesd_sb,
                            presums[side][:, 0:2, :], start=True, stop=True,
                            tile_position=tp)
                # PV: accumulate banded key blocks into C^T [dims, 256].
                # Full-width (N=256) matmuls first so the accumulation group
                # opens with every byte written (PSUM zero-region rule).
                if qb == 0:
                    for mmi in range(2):
                        for side in range(2):
                            tp, hs = tp_hs(side)
                            pAB = pABs[side]
                            vcol = c * 128 + 64 * side
                            if mmi == 0:
                                nc.tensor.matmul(
                                    ps_dcv[hs, 1, :],
                                    v_sb[:, 0, vcol:vcol + 64],
                                    pAB[:, 4:6, :], start=True, stop=False,
                                    tile_position=tp)
                            else:
                                nc.tensor.matmul(
                                    ps_dcv[hs, 1, 128:256],
                                    v_sb[:, 1, vcol:vcol + 64],
                                    pAB[:, 1, :], start=False, stop=True,
                                    tile_position=tp)
                else:
                    jb = 2 * qb
                    for mmi in range(4):
                        for side in range(2):
                            tp, hs = tp_hs(side)
                            pAB = pABs[side]
                            vcol = c * 128 + 64 * side
                            if mmi == 0:
                                nc.tensor.matmul(
                                    ps_dcv[hs, 1, :],
                                    v_sb[:, jb - 1, vcol:vcol + 64],
                                    pAB[:, 2:4, :], start=True, stop=False,
                                    tile_position=tp)
                            elif mmi == 1:
                                nc.tensor.matmul(
                                    ps_dcv[hs, 1, :],
                                    v_sb[:, jb, vcol:vcol + 64],
                                    pAB[:, 4:6, :], start=False, stop=False,
                                    tile_position=tp)
                            elif mmi == 2:
                                nc.tensor.matmul(
                                    ps_dcv[hs, 1, 0:128],
                                    v_sb[:, jb - 2, vcol:vcol + 64],
                                    pAB[:, 0, :], start=False, stop=False,
                                    tile_position=tp)
                            else:
                                nc.tensor.matmul(
                                    ps_dcv[hs, 1, 128:256],
                                    v_sb[:, jb + 1, vcol:vcol + 64],
                                    pAB[:, 1, :], start=False, stop=True,
                                    tile_position=tp)
                recip = work.tile([128, QB], F32, tag="recip", bufs=2,
                                  name=f"recip_{qb}_{c}")
                nc.vector.reciprocal_approx_fast(out=recip,
                                                 in_=ps_dcv[:, 0, :])
                nc.vector.tensor_mul(out=cn, in0=ps_dcv[:, 1, :], in1=recip)
                qb_tails_left[qb] -= 1
                if qb_tails_left[qb] == 0:
                    emit_wo(qb)

            def emit_q(qb, c, qrots):
                qsl = slice(qb * QB, (qb + 1) * QB)
                xq = xt_sb[:, :, qsl]
                ps_q = ps.tile([128, QB], F32, tag="pj", bufs=2,
                               name=f"ps_q_{qb}_{c}")
                for kc in range(NKC):
                    nc.tensor.matmul(
                        ps_q,
                        wq_sb[:, kc, c * 128:(c + 1) * 128],
                        xq[:, kc, :],
                        start=(kc == 0), stop=(kc == NKC - 1),
                    )
                qraw = work.tile([128, QB], MDT, tag="kraw", bufs=4,
                                 name=f"qraw_{qb}_{c}")
                nc.scalar.activation(
                    out=qraw, in_=ps_q,
                    func=mybir.ActivationFunctionType.Identity,
                    bias=bqc_sb[:, c:c + 1])
                ps_rh = ps.tile([128, QB], F32, tag="dc", bufs=2,
                                name=f"ps_rh_{qb}_{c}")
                nc.tensor.matmul(ps_rh, rt_sb, qraw, start=True,
                                 stop=True)
                qrot = work.tile([128, QB], MDT, tag="qrot",
                                 name=f"qrot_{qb}_{c}", bufs=5)
                rhs_sin = work.tile([128, QB], MDT, tag="rhsin", bufs=3,
                                    name=f"rhs_sin_{qb}_{c}")
                nc.vector.tensor_mul(out=rhs_sin, in0=ps_rh,
                                     in1=sin2_sb[:, qsl])
                nc.vector.tensor_mul(out=qraw, in0=qraw,
                                     in1=cos2_sb[:, qsl])
                nc.vector.tensor_add(out=qrot, in0=qraw, in1=rhs_sin)
                qrots.append(qrot)

            def emit_attn(qb, c, qrots):
                cn = work.tile([128, QB], MDT, tag="cn", bufs=10,
                               name=f"cn_{qb}_{c}")
                cn_by_qb[qb].append(cn)
                jb = 2 * qb
                pABs = []
                presums = []
                q2 = qrots[c]
                sABs = []
                for side in range(2):
                    sABs.append(ps_s.tile([128, 6, 128], F32, tag="sAB",
                                          bufs=2,
                                          name=f"sAB_{qb}_{c}_{side}"))
                # Score matmuls interleaved A/B: the two sides contract over
                # disjoint 64-row groups, so adjacent MMs run concurrently.
                if qb == 0:
                    for mmi in range(2):
                        for side in range(2):
                            hs = slice(64 * side, 64 * side + 64)
                            if mmi == 0:
                                nc.tensor.matmul(
                                    sABs[side][:, 4:6, :],
                                    kT_sb[hs, c, 0:128],
                                    q2[hs, :], start=True, stop=True)
                            else:
                                nc.tensor.matmul(
                                    sABs[side][:, 1, :],
                                    kT_sb[hs, c, 128:256],
                                    q2[hs, 128:256], start=True, stop=True)
                else:
                    for mmi in range(4):
                        for side in range(2):
                            hs = slice(64 * side, 64 * side + 64)
                            sAB = sABs[side]
                            if mmi == 0:
                                nc.tensor.matmul(
                                    sAB[:, 2:4, :],
                                    kT_sb[hs, c, (jb - 1) * 128:jb * 128],
                                    q2[hs, :], start=True, stop=True)
                            elif mmi == 1:
                                nc.tensor.matmul(
                                    sAB[:, 4:6, :],
                                    kT_sb[hs, c, jb * 128:(jb + 1) * 128],
                                    q2[hs, :], start=True, stop=True)
                            elif mmi == 2:
                                nc.tensor.matmul(
                                    sAB[:, 0, :],
                                    kT_sb[hs, c, (jb - 2) * 128:(jb - 1) * 128],
                                    q2[hs, 0:128], start=True, stop=True)
                            else:
                                nc.tensor.matmul(
                                    sAB[:, 1, :],
                                    kT_sb[hs, c, (jb + 1) * 128:(jb + 2) * 128],
                                    q2[hs, 128:256], start=True, stop=True)
                for side in range(2):
                    sAB = sABs[side]
                    pAB = attn.tile([128, 6, 128], MDT, tag="pAB",
                                    bufs=12,
                                    name=f"pAB_{qb}_{c}_{side}")
                    if qb == 0:
                        nc.scalar.activation(
                            out=pAB[:, 4:6, :], in_=sAB[:, 4:6, :],
                            func=mybir.ActivationFunctionType.Exp,
                            scale=SCALE)
                        nc.scalar.activation(
                            out=pAB[:, 1, :], in_=sAB[:, 1, :],
                            func=mybir.ActivationFunctionType.Exp,
                            scale=SCALE)
                    else:
                        nc.scalar.activation(
                            out=pAB[:, 0:6, :], in_=sAB[:, 0:6, :],
                            func=mybir.ActivationFunctionType.Exp,
                            scale=SCALE)
                    # masks: left planes (0,3) and diag planes (1,4)
                    if qb > 0:
                        pv4 = pAB[:, 0:6, :].rearrange(
                            "p (a b) f -> p a b f", b=3)[:, :, 0:2, :]
                        nc.vector.tensor_mul(
                            out=pv4, in0=pv4,
                            in1=mask4_sb[:, 0:4, :].rearrange(
                                "p (a b) f -> p a b f", b=2))
                    else:
                        nc.vector.tensor_mul(
                            out=pAB[:, 1:5:3, :],
                            in0=pAB[:, 1:5:3, :], in1=mask4_sb[:, 1:4:2, :])
                    # presum the 3 key-block planes (GpSimd)
                    presum = work.tile([128, 2, 128], MDT, tag="psum",
                                       bufs=12,
                                       name=f"presum_{qb}_{c}_{side}")
                    if qb == 0:
                        nc.gpsimd.tensor_add(
                            out=presum[:, 1, :], in0=pAB[:, 1, :],
                            in1=pAB[:, 5, :])
                    elif qb < NQB - 1:
                        nc.gpsimd.tensor_add(
                            out=presum, in0=pAB[:, 0:2, :],
                            in1=pAB[:, 2:4, :])
                        nc.gpsimd.tensor_add(
                            out=presum, in0=presum, in1=pAB[:, 4:6, :])
                    pABs.append(pAB)
                    presums.append(presum)
                pending.append((qb, c, pABs, presums, cn))
                # qb0 tails need v_sb[0:2], which V(0) only writes during
                # the qb=1 slot - hold all qb0 tails until then
                if len(pending) > DEPTH and qb > 0:
                    emit_tail()

            # ---------- interleaved schedule --------------------------------
            # A(n8) = K(n8) + Q(n8-2) + V(n8-1); B(qb) right after its Q.
            # The bursty A-phase between attention bursts gives the PE a
            # ~4.7us buffer that hides the exp/cn chains on Scalar/DVE.
            for c in range(PAIRS):
                emit_k_pair(0, c)
            for c in range(PAIRS):
                emit_k_pair(1, c)
            for qb in range(NQB):
                if qb + 2 <= NQB - 1:
                    n8 = qb + 2
                    for c in range(PAIRS):
                        emit_k_pair(n8, c)
                        emit_q(qb, c, qrots_by_qb.setdefault(qb, []))
                    if n8 == 3:
                        # V(0..2) here so wv can load late in the startup
                        # window without stalling the PE
                        emit_v(0)
                        emit_v(1)
                        emit_v(2)
                    elif n8 >= 4:
                        emit_v(n8 - 1)
                else:
                    # mini-slots: Q6 (+ V7 + K-rot flush) and Q7
                    for c in range(PAIRS):
                        emit_q(qb, c, qrots_by_qb.setdefault(qb, []))
                    if qb == NQB - 2:
                        while a_pend:
                            emit_ktail()
                        emit_v(NQB - 1)
                qb_tails_left[qb] = PAIRS
                cn_by_qb[qb] = []
                qrots = qrots_by_qb[qb]
                for c in range(PAIRS):
                    emit_attn(qb, c, qrots)
                    if qb == NQB - 1:
                        # drain the tail pipeline gradually so the DVE
                        # recip/cn chain keeps up and the final
                        # out-projection isn't serialized behind 3 tails
                        while len(pending) > max(1, DEPTH - 1 - c):
                            emit_tail()
            while pending:
                emit_tail()

    nc.compile()
    return nc


def _host_consts():
    R64 = np.zeros((64, 64), np.float32)
    for d in range(32):
        R64[d, d + 32] = -1.0
    for d in range(32, 64):
        R64[d, d - 32] = 1.0
    Rblk = np.zeros((128, 128), np.float32)
    Rblk[:64, :64] = R64
    Rblk[64:, 64:] = R64
    rt = np.ascontiguousarray(Rblk.T)

    pv, fv = np.meshgrid(np.arange(128), np.arange(128), indexing="ij")
    maskl = (fv <= pv).astype(np.float32)   # left block: valid q <= k
    maskd = (fv >= pv).astype(np.float32)   # diag block: valid k <= q
    mask4 = np.stack([maskl, maskd, maskl, maskd], axis=1)
    return rt, mask4


def _make_in_maps(x, cos, sin, Wq, bq, Wk, Wv, Wo):
    import ml_dtypes
    mdt_np = np.dtype(ml_dtypes.bfloat16)
    rt, mask4 = _host_consts()
    onesd = np.ones((128, 64), mdt_np)
    in_maps = []
    for core in range(8):
        b, g = core // 2, core % 2
        gs = slice(g * W, (g + 1) * W)
        cosT = np.ascontiguousarray(cos[b].T)
        sinT = np.ascontiguousarray(sin[b].T)
        xT = x[b].T.astype(mdt_np)  # [1024, 2048]
        xtc = np.ascontiguousarray(
            xT.reshape(8, 128, 8, 256).transpose(2, 1, 0, 3))

        def wchunks(Wm):
            # [4, 128, NKC, 128]: per head-pair contiguous DMA chunks
            wfull = Wm[:, gs].reshape(8, 128, 512).transpose(1, 0, 2)
            return np.ascontiguousarray(
                wfull.reshape(128, 8, 4, 128).transpose(2, 0, 1, 3)
            ).astype(mdt_np)

        in_maps.append({
            "xtc": xtc,
            "wq": wchunks(Wq),
            "wk": wchunks(Wk),
            "wv": np.ascontiguousarray(
                Wv[:, gs].reshape(8, 128, 512).transpose(1, 0, 2)
            ).astype(mdt_np),
            "wo": np.ascontiguousarray(
                Wo[gs, :].reshape(4, 128, 1024).transpose(1, 0, 2)
            ).astype(mdt_np),
            "bqc": np.ascontiguousarray(
                bq[gs].reshape(PAIRS, 128).T).astype(np.float32),
            "onesd": onesd,
            "rt": rt.astype(mdt_np),
            "cosh": cosT.astype(mdt_np),
            "sinh": sinT.astype(mdt_np),
            "mask4": mask4.astype(mdt_np),
        })
    return in_maps


def _get_nc():
    global _STATE
    if _STATE is None:
        _STATE = _build()
    return _STATE


def run(inputs, trace=False, trace_cores=None):
    """Run the SPMD kernel; returns (full_output, BassKernelResults)."""
    nc = _get_nc()
    in_maps = _make_in_maps(
        inputs["x"], inputs["cos"], inputs["sin"], inputs["Wq"], inputs["bq"],
        inputs["Wk"], inputs["Wv"], inputs["Wo"])
    res = bass_utils.run_bass_kernel_spmd(
        nc, in_maps, core_ids=list(range(8)), trace=trace,
        trace_cores=trace_cores)
    mask = np.asarray(inputs["mask"])
    bo = np.asarray(inputs["bo"])
    bv = np.asarray(inputs["bv"])
    Wo = np.asarray(inputs["Wo"])
    out = np.zeros((B, S, E), np.float32)
    for core in range(8):
        b = core // 2
        out[b] += res.results[core]["out"]
    out += (bv @ Wo + bo)[None, None, :]
    out *= mask[..., None].astype(np.float32)
    return out, res


def kernel(**inputs) -> np.ndarray:
    inputs = {k: np.asarray(v) for k, v in inputs.items()}
    out, _ = run(inputs)
    return out



# revision 55
# speedup vs baseline: 1.0008x; 1.0008x over previous
"""Trainium2 Bass kernel for windowed causal multi-head attention.

Problem (hardcoded): x [4, 2048, 1024], 16 heads x 64 dim, rotary embedding,
causal attention with left window 256, fused QKV/out projections.

Sharding: 8 cores = (batch b in 0..3) x (head-group g in 0..1). Each core
computes batch b, heads [g*8, (g+1)*8) and produces a partial output
[2048, 1024] (its head-group's contribution to the out-projection). The host
sums the two partials per batch and adds the output bias (which also absorbs
bv: softmax weights sum to 1, so v+bv contributes bv@Wo to every output row).

Device-side layout strategy (transpose-free, all matmuls bf16):
  - Projections compute qT/kT [hd, seq] with head-dim on partitions
    (lhsT = W chunk) and v [seq, hd] naturally (lhsT = xT chunk). Rotary
    rotate_half is a 128x128 constant matmul plus two elementwise multiplies
    against host-precomputed cos/sin rows.
  - Per (256-query block, head pair, side): scores S^T [keys, queries] are
    computed as 4 matmuls over the 4 banded key blocks; the two middle key
    blocks are shared by both 128-query halves and run at N=256. Layout is
    a [128, 6, 128] PSUM tile: plane m = 2*jb_rel + query_half, so the
    half-planes needed for masks/presum/PV fall on regular strides.
  - Softmax: one exp per side (no max subtraction: |scores| < ~10), band
    masks as two {0,1}-multiplies on strided plane pairs (DVE), then the
    3 key-block planes are pre-summed on GpSimd so each side's softmax
    denominator is a single ones-matmul at N=256 (PE cost 1/3 of per-block
    denominator matmuls). Context C^T accumulates the banded key blocks with
    the same N=256 middle-block sharing; normalized with a reciprocal
    multiply.
  - Out projection consumes C^T directly as lhsT (K = head dims).
  - Stage A (K/V projections) and stage B (Q + attention + out-proj) are
    interleaved per 256-position block so Scalar/DVE/GpSimd load spreads.
  - Engines execute queues in order, so emission order is the schedule:
    attention tails lag their scores by DEPTH iterations so the PE never
    waits on the exp->mask->presum chain.
"""

import numpy as np

import concourse.bass as bass
import concourse.mybir as mybir
import concourse.tile as tile
from concourse import bacc
from concourse import bass_utils

B, S, E = 4, 2048, 1024
H, D = 16, 64
W = 512          # per-core head-group width (8 heads x 64)
QB = 256         # query block
NQB = S // QB    # 8
NKC = E // 128   # 8 contraction chunks for projections
PAIRS = 4        # head pairs per core (128 cols each)
SCALE = 1.0 / 8.0  # 1/sqrt(D)
DEPTH = 4        # attention-tail pipeline depth (in (qb,c) iterations)

F32 = mybir.dt.float32
MDT = mybir.dt.bfloat16

_STATE = None


def _build():
    nc = bacc.Bacc("TRN2", target_bir_lowering=False, debug=False, num_devices=8)

    xtc = nc.dram_tensor("xtc", [NQB, 128, NKC, QB], MDT,
                         kind="ExternalInput").ap()
    wq = nc.dram_tensor("wq", [PAIRS, 128, NKC, 128], MDT,
                        kind="ExternalInput").ap()
    wk = nc.dram_tensor("wk", [PAIRS, 128, NKC, 128], MDT,
                        kind="ExternalInput").ap()
    wv = nc.dram_tensor("wv", [128, NKC, W], MDT, kind="ExternalInput").ap()
    wo = nc.dram_tensor("wo", [128, PAIRS, E], MDT, kind="ExternalInput").ap()
    bqc = nc.dram_tensor("bqc", [128, PAIRS], F32, kind="ExternalInput").ap()
    onesd = nc.dram_tensor("onesd", [128, 64], MDT, kind="ExternalInput").ap()
    rt = nc.dram_tensor("rt", [128, 128], MDT, kind="ExternalInput").ap()
    cosh = nc.dram_tensor("cosh", [64, S], MDT, kind="ExternalInput").ap()
    sinh = nc.dram_tensor("sinh", [64, S], MDT, kind="ExternalInput").ap()
    mask4 = nc.dram_tensor("mask4", [128, 4, 128], MDT,
                           kind="ExternalInput").ap()
    out = nc.dram_tensor("out", [S, E], F32, kind="ExternalOutput").ap()

    with tile.TileContext(nc) as tc:
        with tc.tile_pool(name="res", bufs=1) as res, \
             tc.tile_pool(name="work", bufs=2) as work, \
             tc.tile_pool(name="attn", bufs=2) as attn, \
             tc.tile_pool(name="ps", bufs=4, space="PSUM") as ps, \
             tc.tile_pool(name="ps_s", bufs=2, space="PSUM") as ps_s:

            # --- resident constants / weights ---
            xt_sb = res.tile([128, NKC, S], MDT)     # resident x^T
            cos2_sb = res.tile([128, S], MDT)
            sin2_sb = res.tile([128, S], MDT)
            wq_sb = res.tile([128, NKC, W], MDT)
            wk_sb = res.tile([128, NKC, W], MDT)
            wv_sb = res.tile([128, NKC, W], MDT)
            wo_sb = res.tile([128, PAIRS, E], MDT)
            bqc_sb = res.tile([128, PAIRS], F32)
            onesd_sb = res.tile([128, 64], MDT)
            rt_sb = res.tile([128, 128], MDT)
            mask4_sb = res.tile([128, 4, 128], MDT)
            kT_sb = res.tile([128, PAIRS, S], MDT)   # rotated K^T per pair
            v_sb = res.tile([128, S // 128, W], MDT)  # V (seq-major tiles)

            # Startup DMAs: HBM pull runs at a fixed ~260GB/s regardless of
            # queue count, so a single sync queue in strict consumption
            # order is the best schedule. Early bytes are minimized: cos/sin
            # are loaded once for positions 0:1024 (blocks 0-3) and
            # duplicated to the upper partition half SBUF->SBUF on scalar;
            # the 1024:2048 halves ride at the end of the chain.
            # first halves split so pair0's first matmuls can start ~3us
            # earlier while the rest streams in
            nc.sync.dma_start(out=wk_sb[:, 0:4, 0:128], in_=wk[0][:, 0:4, :])
            nc.sync.dma_start(out=xt_sb[:, 0:4, 0:QB], in_=xtc[0][:, 0:4, :])
            nc.sync.dma_start(out=wk_sb[:, 4:8, 0:128], in_=wk[0][:, 4:8, :])
            nc.sync.dma_start(out=xt_sb[:, 4:8, 0:QB], in_=xtc[0][:, 4:8, :])
            nc.sync.dma_start(out=wk_sb[:, :, 128:256], in_=wk[1])
            nc.sync.dma_start(out=wk_sb[:, :, 256:384], in_=wk[2])
            nc.sync.dma_start(out=wk_sb[:, :, 384:512], in_=wk[3])
            nc.sync.dma_start(out=rt_sb, in_=rt)
            nc.sync.dma_start(out=cos2_sb[0:64, 0:1024], in_=cosh[:, 0:1024])
            nc.sync.dma_start(out=sin2_sb[0:64, 0:1024], in_=sinh[:, 0:1024])
            nc.sync.dma_start(out=xt_sb[:, :, QB:2 * QB], in_=xtc[1])
            # dups after xtc1 (their DVE consumers are elastic); same queue
            # as the source loads: DMA->DMA ordering is only guaranteed by
            # queue FIFO
            nc.sync.dma_start(out=cos2_sb[64:128, 0:1024],
                              in_=cos2_sb[0:64, 0:1024])
            nc.sync.dma_start(out=sin2_sb[64:128, 0:1024],
                              in_=sin2_sb[0:64, 0:1024])
            # xtc2 in kc-halves: K(2) can start on the first half while
            # the second is still landing
            nc.sync.dma_start(out=xt_sb[:, 0:4, 2 * QB:3 * QB],
                              in_=xtc[2][:, 0:4, :])
            nc.sync.dma_start(out=xt_sb[:, 4:8, 2 * QB:3 * QB],
                              in_=xtc[2][:, 4:8, :])
            nc.sync.dma_start(out=bqc_sb, in_=bqc)
            for c in range(PAIRS):
                nc.sync.dma_start(out=wq_sb[:, :, c * 128:(c + 1) * 128],
                                  in_=wq[c])
            nc.sync.dma_start(out=xt_sb[:, :, 3 * QB:4 * QB], in_=xtc[3])
            nc.sync.dma_start(out=wv_sb, in_=wv)
            nc.sync.dma_start(out=wo_sb, in_=wo)
            nc.sync.dma_start(out=cos2_sb[0:64, 1024:2048],
                              in_=cosh[:, 1024:2048])
            nc.sync.dma_start(out=sin2_sb[0:64, 1024:2048],
                              in_=sinh[:, 1024:2048])
            nc.sync.dma_start(out=cos2_sb[64:128, 1024:2048],
                              in_=cos2_sb[0:64, 1024:2048])
            nc.sync.dma_start(out=sin2_sb[64:128, 1024:2048],
                              in_=sin2_sb[0:64, 1024:2048])
            # scalar queue: free until its first kraw copy (~13us);
            # DMA->compute deps are tracked across queues, so these can
            # ride here without diluting the sync chain's priority order
            nc.scalar.dma_start(out=mask4_sb, in_=mask4)
            nc.scalar.dma_start(out=onesd_sb, in_=onesd)

            # ---------- Stage A: K^T (rotated) + V for one 256-seq block ----
            a_pend = []
            a_pend_c = []

            def emit_ktail():
                kraw, sl_ = a_pend.pop(0)
                ps_rh = ps.tile([128, QB], F32, tag="dc", bufs=2,
                                name=f"ps_rh_a_{sl_.start}")
                nc.tensor.matmul(ps_rh, rt_sb, kraw, start=True, stop=True)
                rhs_sin = work.tile([128, QB], MDT, tag="rhsin", bufs=3,
                                    name=f"rhs_sin_a_{sl_.start}")
                nc.vector.tensor_mul(out=rhs_sin, in0=ps_rh,
                                     in1=sin2_sb[:, sl_])
                nc.vector.tensor_mul(out=kraw, in0=kraw,
                                     in1=cos2_sb[:, sl_])
                nc.vector.tensor_add(
                    out=kT_sb[:, a_pend_c.pop(0), sl_], in0=kraw,
                    in1=rhs_sin)

            def emit_v(n8):
                sl = slice(n8 * QB, (n8 + 1) * QB)
                xa = xt_sb[:, :, sl]
                for sub in range(2):
                    jb = n8 * 2 + sub
                    ps_v = ps.tile([128, W], F32, tag="pj", bufs=2,
                                   name=f"ps_v_{jb}")
                    for kc in range(NKC):
                        nc.tensor.matmul(
                            ps_v,
                            xa[:, kc, sub * 128:(sub + 1) * 128],
                            wv_sb[:, kc, :],
                            start=(kc == 0), stop=(kc == NKC - 1),
                        )
                    # V copy on DVE: on Scalar it queues ahead of the
                    # exps whose latency gates the sAB score-PSUM ring
                    nc.vector.tensor_copy(out=v_sb[:, jb, :], in_=ps_v)

            def emit_k_pair(n8, c):
                # one K-projection unit: block n8, head pair c
                sl = slice(n8 * QB, (n8 + 1) * QB)
                xa = xt_sb[:, :, sl]
                if c == 0 and 4 <= n8 + 2 <= 7:
                    pre = n8 + 2
                    nc.sync.dma_start(
                        out=xt_sb[:, :, pre * QB:(pre + 1) * QB],
                        in_=xtc[pre])
                ps_k = ps.tile([128, QB], F32, tag="pj", bufs=2,
                               name=f"ps_k_{n8}_{c}")
                for kc in range(NKC):
                    nc.tensor.matmul(
                        ps_k,
                        wk_sb[:, kc, c * 128:(c + 1) * 128],
                        xa[:, kc, :],
                        start=(kc == 0), stop=(kc == NKC - 1),
                    )
                kraw = work.tile([128, QB], MDT, tag="kraw", bufs=4,
                                 name=f"kraw_{n8}_{c}")
                nc.scalar.activation(
                    out=kraw, in_=ps_k,
                    func=mybir.ActivationFunctionType.Copy)
                a_pend.append((kraw, sl))
                a_pend_c.append(c)
                if len(a_pend) > 1:
                    emit_ktail()

            # ---------- Stage B: Q + attention + out-projection -------------
            pending = []          # (qb, c, pABs, presums, cn)
            qb_tails_left = {}
            cn_by_qb = {}
            qrots_by_qb = {}

            def emit_wo(qb):
                cts = cn_by_qb.pop(qb)
                last = qb == NQB - 1
                for sub in range(2):
                    o_sb = work.tile([128, 1024], F32, tag="o_sb",
                                     name=f"o_sb_{qb}_{sub}")
                    rows = slice(qb * QB + sub * 128,
                                 qb * QB + (sub + 1) * 128)
                    for ncol in range(2):
                        # sub1 groups borrow the dc ring so the next
                        # A-burst's first K matmuls (pj ring) don't wait
                        # out-copy latency at the block seam
                        ps_o = ps.tile([128, 512], F32,
                                       tag="pj" if sub == 0 else "dc",
                                       bufs=2,
                                       name=f"ps_o_{qb}_{sub}_{ncol}")
                        for cc in range(PAIRS):
                            nc.tensor.matmul(
                                ps_o,
                                cts[cc][:, sub * 128:(sub + 1) * 128],
                                wo_sb[:, cc, ncol * 512:(ncol + 1) * 512],
                                start=(cc == 0), stop=(cc == PAIRS - 1))
                        osl = slice(ncol * 512, (ncol + 1) * 512)
                        # split copies across Scalar/DVE so exps never queue
                        # behind a burst of out-copies
                        if last:
                            # queues are drained here: halve the copy wall
                            # by splitting across Scalar+DVE
                            nc.scalar.activation(
                                out=o_sb[:, ncol * 512:ncol * 512 + 256],
                                in_=ps_o[:, 0:256],
                                func=mybir.ActivationFunctionType.Copy)
                            nc.vector.tensor_copy(
                                out=o_sb[:, ncol * 512 + 256:(ncol + 1) * 512],
                                in_=ps_o[:, 256:512])
                        elif ncol == 1:
                            nc.vector.tensor_copy(out=o_sb[:, osl],
                                                  in_=ps_o)
                        else:
                            nc.scalar.activation(
                                out=o_sb[:, osl], in_=ps_o,
                                func=mybir.ActivationFunctionType.Copy)
                        if last:
                            eng = (nc.sync, nc.scalar, nc.sync,
                                   nc.gpsimd)[2 * sub + ncol]
                            eng.dma_start(out=out[rows, osl],
                                          in_=o_sb[:, osl])
                    if not last:
                        nc.sync.dma_start(out=out[rows, :], in_=o_sb)

            def emit_tail():
                qb, c, pABs, presums, cn = pending.pop(0)
                ps_dcv = ps.tile([128, 2, QB], F32, tag="dc", bufs=2,
                                 name=f"ps_dcv_{qb}_{c}")
                # Denominators: ones-matmul over the presummed planes.
                # Side A -> partitions 0:64, side B -> 64:128 (tile_position).
                # All den/PV matmuls are emitted A/B-interleaved: the sides
                # occupy disjoint 64-col groups, so adjacent MMs overlap.
                def tp_hs(side):
                    return ((None, (0, 64))[side],
                            slice(64 * side, 64 * side + 64))
                if qb == 0:
                    for mmi in range(2):
                        for side in range(2):
                            tp, hs = tp_hs(side)
                            if mmi == 0:
                                # h0 den = masked diag plane only
                                nc.tensor.matmul(
                                    ps_dcv[hs, 0, 0:128], onesd_sb,
                                    pABs[side][:, 4, :], start=True,
                                    stop=True, tile_position=tp)
                            else:
                                nc.tensor.matmul(
                                    ps_dcv[hs, 0, 128:256], onesd_sb,
                                    presums[side][:, 1, :], start=True,
                                    stop=True, tile_position=tp)
                elif qb == NQB - 1:
                    # Last block: no presum (GpSimd chain would sit on
                    # the drain critical path) - accumulate planes on PE.
                    for half in range(2):
                        csl = slice(half * 128, (half + 1) * 128)
                        for i, m in enumerate((half, half + 2, half + 4)):
                            for side in range(2):
                                tp, hs = tp_hs(side)
                                nc.tensor.matmul(
                                    ps_dcv[hs, 0, csl], onesd_sb,
                                    pABs[side][:, m, :], start=(i == 0),
                                    stop=(i == 2), tile_position=tp)
                else:
                    for side in range(2):
                        tp, hs = tp_hs(side)
                        nc.tensor.matmul(
                            ps_dcv[hs, 0, :], onesd_sb,
                            presums[side][:, 0:2, :], start=True, stop=True,
                            tile_position=tp)
                # PV: accumulate banded key blocks into C^T [dims, 256].
                # Full-width (N=256) matmuls first so the accumulation group
                # opens with every byte written (PSUM zero-region rule).
                if qb == 0:
                    for mmi in range(2):
                        for side in range(2):
                            tp, hs = tp_hs(side)
                            pAB = pABs[side]
                            vcol = c * 128 + 64 * side
                            if mmi == 0:
                                nc.tensor.matmul(
                                    ps_dcv[hs, 1, :],
                                    v_sb[:, 0, vcol:vcol + 64],
                                    pAB[:, 4:6, :], start=True, stop=False,
                                    tile_position=tp)
                            else:
                                nc.tensor.matmul(
                                    ps_dcv[hs, 1, 128:256],
                                    v_sb[:, 1, vcol:vcol + 64],
                                    pAB[:, 1, :], start=False, stop=True,
                                    tile_position=tp)
                else:
                    jb = 2 * qb
                    for mmi in range(4):
                        for side in range(2):
                            tp, hs = tp_hs(side)
                            pAB = pABs[side]
                            vcol = c * 128 + 64 * side
                            if mmi == 0:
                                nc.tensor.matmul(
                                    ps_dcv[hs, 1, :],
                                    v_sb[:, jb - 1, vcol:vcol + 64],
                                    pAB[:, 2:4, :], start=True, stop=False,
                                    tile_position=tp)
                            elif mmi == 1:
                                nc.tensor.matmul(
                                    ps_dcv[hs, 1, :],
                                    v_sb[:, jb, vcol:vcol + 64],
                                    pAB[:, 4:6, :], start=False, stop=False,
                                    tile_position=tp)
                            elif mmi == 2:
                                nc.tensor.matmul(
                                    ps_dcv[hs, 1, 0:128],
                                    v_sb[:, jb - 2, vcol:vcol + 64],
                                    pAB[:, 0, :], start=False, stop=False,
                                    tile_position=tp)
                            else:
                                nc.tensor.matmul(
                                    ps_dcv[hs, 1, 128:256],
                                    v_sb[:, jb + 1, vcol:vcol + 64],
                                    pAB[:, 1, :], start=False, stop=True,
                                    tile_position=tp)
                recip = work.tile([128, QB], F32, tag="recip", bufs=2,
                                  name=f"recip_{qb}_{c}")
                nc.vector.reciprocal_approx_fast(out=recip,
                                                 in_=ps_dcv[:, 0, :])
                nc.vector.tensor_mul(out=cn, in0=ps_dcv[:, 1, :], in1=recip)
                qb_tails_left[qb] -= 1
                if qb_tails_left[qb] == 0:
                    emit_wo(qb)

            def emit_q(qb, c, qrots):
                qsl = slice(qb * QB, (qb + 1) * QB)
                xq = xt_sb[:, :, qsl]
                ps_q = ps.tile([128, QB], F32, tag="pj", bufs=2,
                               name=f"ps_q_{qb}_{c}")
                for kc in range(NKC):
                    nc.tensor.matmul(
                        ps_q,
                        wq_sb[:, kc, c * 128:(c + 1) * 128],
                        xq[:, kc, :],
                        start=(kc == 0), stop=(kc == NKC - 1),
                    )
                qraw = work.tile([128, QB], MDT, tag="kraw", bufs=4,
                                 name=f"qraw_{qb}_{c}")
                nc.scalar.activation(
                    out=qraw, in_=ps_q,
                    func=mybir.ActivationFunctionType.Identity,
                    bias=bqc_sb[:, c:c + 1])
                ps_rh = ps.tile([128, QB], F32, tag="dc", bufs=2,
                                name=f"ps_rh_{qb}_{c}")
                nc.tensor.matmul(ps_rh, rt_sb, qraw, start=True,
                                 stop=True)
                qrot = work.tile([128, QB], MDT, tag="qrot",
                                 name=f"qrot_{qb}_{c}", bufs=5)
                rhs_sin = work.tile([128, QB], MDT, tag="rhsin", bufs=3,
                                    name=f"rhs_sin_{qb}_{c}")
                nc.vector.tensor_mul(out=rhs_sin, in0=ps_rh,
                                     in1=sin2_sb[:, qsl])
                nc.vector.tensor_mul(out=qraw, in0=qraw,
                                     in1=cos2_sb[:, qsl])
                nc.vector.tensor_add(out=qrot, in0=qraw, in1=rhs_sin)
                qrots.append(qrot)

            def emit_attn(qb, c, qrots):
                cn = work.tile([128, QB], MDT, tag="cn", bufs=10,
                               name=f"cn_{qb}_{c}")
                cn_by_qb[qb].append(cn)
                jb = 2 * qb
                pABs = []
                presums = []
                q2 = qrots[c]
                sABs = []
                for side in range(2):
                    sABs.append(ps_s.tile([128, 6, 128], F32, tag="sAB",
                                          bufs=2,
                                          name=f"sAB_{qb}_{c}_{side}"))
                # Score matmuls interleaved A/B: the two sides contract over
                # disjoint 64-row groups, so adjacent MMs run concurrently.
                if qb == 0:
                    for mmi in range(2):
                        for side in range(2):
                            hs = slice(64 * side, 64 * side + 64)
                            if mmi == 0:
                                nc.tensor.matmul(
                                    sABs[side][:, 4:6, :],
                                    kT_sb[hs, c, 0:128],
                                    q2[hs, :], start=True, stop=True)
                            else:
                                nc.tensor.matmul(
                                    sABs[side][:, 1, :],
                                    kT_sb[hs, c, 128:256],
                                    q2[hs, 128:256], start=True, stop=True)
                else:
                    for mmi in range(4):
                        for side in range(2):
                            hs = slice(64 * side, 64 * side + 64)
                            sAB = sABs[side]
                            if mmi == 0:
                                nc.tensor.matmul(
                                    sAB[:, 2:4, :],
                                    kT_sb[hs, c, (jb - 1) * 128:jb * 128],
                                    q2[hs, :], start=True, stop=True)
                            elif mmi == 1:
                                nc.tensor.matmul(
                                    sAB[:, 4:6, :],
                                    kT_sb[hs, c, jb * 128:(jb + 1) * 128],
                                    q2[hs, :], start=True, stop=True)
                            elif mmi == 2:
                                nc.tensor.matmul(
                                    sAB[:, 0, :],
                                    kT_sb[hs, c, (jb - 2) * 128:(jb - 1) * 128],
                                    q2[hs, 0:128], start=True, stop=True)
                            else:
                                nc.tensor.matmul(
                                    sAB[:, 1, :],
                                    kT_sb[hs, c, (jb + 1) * 128:(jb + 2) * 128],
                                    q2[hs, 128:256], start=True, stop=True)
                for side in range(2):
                    sAB = sABs[side]
                    pAB = attn.tile([128, 6, 128], MDT, tag="pAB",
                                    bufs=12,
                                    name=f"pAB_{qb}_{c}_{side}")
                    if qb == 0:
                        nc.scalar.activation(
                            out=pAB[:, 4:6, :], in_=sAB[:, 4:6, :],
                            func=mybir.ActivationFunctionType.Exp,
                            scale=SCALE)
                        nc.scalar.activation(
                            out=pAB[:, 1, :], in_=sAB[:, 1, :],
                            func=mybir.ActivationFunctionType.Exp,
                            scale=SCALE)
                    else:
                        nc.scalar.activation(
                            out=pAB[:, 0:6, :], in_=sAB[:, 0:6, :],
                            func=mybir.ActivationFunctionType.Exp,
                            scale=SCALE)
                    # masks: left planes (0,3) and diag planes (1,4)
                    if qb > 0:
                        pv4 = pAB[:, 0:6, :].rearrange(
                            "p (a b) f -> p a b f", b=3)[:, :, 0:2, :]
                        nc.vector.tensor_mul(
                            out=pv4, in0=pv4,
                            in1=mask4_sb[:, 0:4, :].rearrange(
                                "p (a b) f -> p a b f", b=2))
                    else:
                        nc.vector.tensor_mul(
                            out=pAB[:, 1:5:3, :],
                            in0=pAB[:, 1:5:3, :], in1=mask4_sb[:, 1:4:2, :])
                    # presum the 3 key-block planes (GpSimd)
                    presum = work.tile([128, 2, 128], MDT, tag="psum",
                                       bufs=12,
                                       name=f"presum_{qb}_{c}_{side}")
                    if qb == 0:
                        nc.gpsimd.tensor_add(
                            out=presum[:, 1, :], in0=pAB[:, 1, :],
                            in1=pAB[:, 5, :])
                    elif qb < NQB - 1:
                        nc.gpsimd.tensor_add(
                            out=presum, in0=pAB[:, 0:2, :],
                            in1=pAB[:, 2:4, :])
                        nc.gpsimd.tensor_add(
                            out=presum, in0=presum, in1=pAB[:, 4:6, :])
                    pABs.append(pAB)
                    presums.append(presum)
                pending.append((qb, c, pABs, presums, cn))
                # qb0 tails need v_sb[0:2], which V(0) only writes during
                # the qb=1 slot - hold all qb0 tails until then
                if len(pending) > DEPTH and qb > 0:
                    emit_tail()

            # ---------- interleaved schedule --------------------------------
            # A(n8) = K(n8) + Q(n8-2) + V(n8-1); B(qb) right after its Q.
            # The bursty A-phase between attention bursts gives the PE a
            # ~4.7us buffer that hides the exp/cn chains on Scalar/DVE.
            for c in range(PAIRS):
                emit_k_pair(0, c)
            for c in range(PAIRS):
                emit_k_pair(1, c)
            for qb in range(NQB):
                if qb + 2 <= NQB - 1:
                    n8 = qb + 2
                    for c in range(PAIRS):
                        emit_k_pair(n8, c)
                        emit_q(qb, c, qrots_by_qb.setdefault(qb, []))
                    if n8 == 3:
                        # V(0..2) here so wv can load late in the startup
                        # window without stalling the PE
                        emit_v(0)
                        emit_v(1)
                        emit_v(2)
                    elif n8 >= 4:
                        emit_v(n8 - 1)
                else:
                    # mini-slots: Q6 (+ V7 + K-rot flush) and Q7
                    for c in range(PAIRS):
                        emit_q(qb, c, qrots_by_qb.setdefault(qb, []))
                    if qb == NQB - 2:
                        while a_pend:
                            emit_ktail()
                        emit_v(NQB - 1)
                qb_tails_left[qb] = PAIRS
                cn_by_qb[qb] = []
                qrots = qrots_by_qb[qb]
                for c in range(PAIRS):
                    emit_attn(qb, c, qrots)
                    if qb == NQB - 1:
                        # drain the tail pipeline gradually so the DVE
                        # recip/cn chain keeps up and the final
                        # out-projection isn't serialized behind 3 tails
                        while len(pending) > max(1, DEPTH - 1 - c):
                            emit_tail()
            while pending:
                emit_tail()

    nc.compile()
    return nc


def _host_consts():
    R64 = np.zeros((64, 64), np.float32)
    for d in range(32):
        R64[d, d + 32] = -1.0
    for d in range(32, 64):
        R64[d, d - 32] = 1.0
    Rblk = np.zeros((128, 128), np.float32)
    Rblk[:64, :64] = R64
    Rblk[64:, 64:] = R64
    rt = np.ascontiguousarray(Rblk.T)

    pv, fv = np.meshgrid(np.arange(128), np.arange(128), indexing="ij")
    maskl = (fv <= pv).astype(np.float32)   # left block: valid q <= k
    maskd = (fv >= pv).astype(np.float32)   # diag block: valid k <= q
    mask4 = np.stack([maskl, maskd, maskl, maskd], axis=1)
    return rt, mask4


def _make_in_maps(x, cos, sin, Wq, bq, Wk, Wv, Wo):
    import ml_dtypes
    mdt_np = np.dtype(ml_dtypes.bfloat16)
    rt, mask4 = _host_consts()
    onesd = np.ones((128, 64), mdt_np)
    in_maps = []
    for core in range(8):
        b, g = core // 2, core % 2
        gs = slice(g * W, (g + 1) * W)
        cosT = np.ascontiguousarray(cos[b].T)
        sinT = np.ascontiguousarray(sin[b].T)
        xT = x[b].T.astype(mdt_np)  # [1024, 2048]
        xtc = np.ascontiguousarray(
            xT.reshape(8, 128, 8, 256).transpose(2, 1, 0, 3))

        def wchunks(Wm):
            # [4, 128, NKC, 128]: per head-pair contiguous DMA chunks
            wfull = Wm[:, gs].reshape(8, 128, 512).transpose(1, 0, 2)
            return np.ascontiguousarray(
                wfull.reshape(128, 8, 4, 128).transpose(2, 0, 1, 3)
            ).astype(mdt_np)

        in_maps.append({
            "xtc": xtc,
            "wq": wchunks(Wq),
            "wk": wchunks(Wk),
            "wv": np.ascontiguousarray(
                Wv[:, gs].reshape(8, 128, 512).transpose(1, 0, 2)
            ).astype(mdt_np),
            "wo": np.ascontiguousarray(
                Wo[gs, :].reshape(4, 128, 1024).transpose(1, 0, 2)
            ).astype(mdt_np),
            "bqc": np.ascontiguousarray(
                bq[gs].reshape(PAIRS, 128).T).astype(np.float32),
            "onesd": onesd,
            "rt": rt.astype(mdt_np),
            "cosh": cosT.astype(mdt_np),
            "sinh": sinT.astype(mdt_np),
            "mask4": mask4.astype(mdt_np),
        })
    return in_maps


def _get_nc():
    global _STATE
    if _STATE is None:
        _STATE = _build()
    return _STATE


def run(inputs, trace=False, trace_cores=None):
    """Run the SPMD kernel; returns (full_output, BassKernelResults)."""
    nc = _get_nc()
    in_maps = _make_in_maps(
        inputs["x"], inputs["cos"], inputs["sin"], inputs["Wq"], inputs["bq"],
        inputs["Wk"], inputs["Wv"], inputs["Wo"])
    res = bass_utils.run_bass_kernel_spmd(
        nc, in_maps, core_ids=list(range(8)), trace=trace,
        trace_cores=trace_cores)
    mask = np.asarray(inputs["mask"])
    bo = np.asarray(inputs["bo"])
    bv = np.asarray(inputs["bv"])
    Wo = np.asarray(inputs["Wo"])
    out = np.zeros((B, S, E), np.float32)
    for core in range(8):
        b = core // 2
        out[b] += res.results[core]["out"]
    out += (bv @ Wo + bo)[None, None, :]
    out *= mask[..., None].astype(np.float32)
    return out, res


def kernel(**inputs) -> np.ndarray:
    inputs = {k: np.asarray(v) for k, v in inputs.items()}
    out, _ = run(inputs)
    return out



# revision 56
# speedup vs baseline: 1.0051x; 1.0043x over previous
"""Trainium2 Bass kernel for windowed causal multi-head attention.

Problem (hardcoded): x [4, 2048, 1024], 16 heads x 64 dim, rotary embedding,
causal attention with left window 256, fused QKV/out projections.

Sharding: 8 cores = (batch b in 0..3) x (head-group g in 0..1). Each core
computes batch b, heads [g*8, (g+1)*8) and produces a partial output
[2048, 1024] (its head-group's contribution to the out-projection). The host
sums the two partials per batch and adds the output bias (which also absorbs
bv: softmax weights sum to 1, so v+bv contributes bv@Wo to every output row).

Device-side layout strategy (transpose-free, all matmuls bf16):
  - Projections compute qT/kT [hd, seq] with head-dim on partitions
    (lhsT = W chunk) and v [seq, hd] naturally (lhsT = xT chunk). Rotary
    rotate_half is a 128x128 constant matmul plus two elementwise multiplies
    against host-precomputed cos/sin rows.
  - Per (256-query block, head pair, side): scores S^T [keys, queries] are
    computed as 4 matmuls over the 4 banded key blocks; the two middle key
    blocks are shared by both 128-query halves and run at N=256. Layout is
    a [128, 6, 128] PSUM tile: plane m = 2*jb_rel + query_half, so the
    half-planes needed for masks/presum/PV fall on regular strides.
  - Softmax: one exp per side (no max subtraction: |scores| < ~10), band
    masks as two {0,1}-multiplies on strided plane pairs (DVE), then the
    3 key-block planes are pre-summed on GpSimd so each side's softmax
    denominator is a single ones-matmul at N=256 (PE cost 1/3 of per-block
    denominator matmuls). Context C^T accumulates the banded key blocks with
    the same N=256 middle-block sharing; normalized with a reciprocal
    multiply.
  - Out projection consumes C^T directly as lhsT (K = head dims).
  - Stage A (K/V projections) and stage B (Q + attention + out-proj) are
    interleaved per 256-position block so Scalar/DVE/GpSimd load spreads.
  - Engines execute queues in order, so emission order is the schedule:
    attention tails lag their scores by DEPTH iterations so the PE never
    waits on the exp->mask->presum chain.
"""

import numpy as np

import concourse.bass as bass
import concourse.mybir as mybir
import concourse.tile as tile
from concourse import bacc
from concourse import bass_utils

B, S, E = 4, 2048, 1024
H, D = 16, 64
W = 512          # per-core head-group width (8 heads x 64)
QB = 256         # query block
NQB = S // QB    # 8
NKC = E // 128   # 8 contraction chunks for projections
PAIRS = 4        # head pairs per core (128 cols each)
SCALE = 1.0 / 8.0  # 1/sqrt(D)
DEPTH = 4        # attention-tail pipeline depth (in (qb,c) iterations)

F32 = mybir.dt.float32
MDT = mybir.dt.bfloat16

_STATE = None


def _build():
    nc = bacc.Bacc("TRN2", target_bir_lowering=False, debug=False, num_devices=8)

    xtc = nc.dram_tensor("xtc", [NQB, 128, NKC, QB], MDT,
                         kind="ExternalInput").ap()
    wq = nc.dram_tensor("wq", [PAIRS, 128, NKC, 128], MDT,
                        kind="ExternalInput").ap()
    wk = nc.dram_tensor("wk", [PAIRS, 128, NKC, 128], MDT,
                        kind="ExternalInput").ap()
    wv = nc.dram_tensor("wv", [128, NKC, W], MDT, kind="ExternalInput").ap()
    wo = nc.dram_tensor("wo", [128, PAIRS, E], MDT, kind="ExternalInput").ap()
    bqc = nc.dram_tensor("bqc", [128, PAIRS], F32, kind="ExternalInput").ap()
    onesd = nc.dram_tensor("onesd", [128, 64], MDT, kind="ExternalInput").ap()
    rt = nc.dram_tensor("rt", [128, 128], MDT, kind="ExternalInput").ap()
    cosh = nc.dram_tensor("cosh", [64, S], MDT, kind="ExternalInput").ap()
    sinh = nc.dram_tensor("sinh", [64, S], MDT, kind="ExternalInput").ap()
    mask4 = nc.dram_tensor("mask4", [128, 4, 128], MDT,
                           kind="ExternalInput").ap()
    out = nc.dram_tensor("out", [S, E], F32, kind="ExternalOutput").ap()

    with tile.TileContext(nc) as tc:
        with tc.tile_pool(name="res", bufs=1) as res, \
             tc.tile_pool(name="work", bufs=2) as work, \
             tc.tile_pool(name="attn", bufs=2) as attn, \
             tc.tile_pool(name="ps", bufs=4, space="PSUM") as ps, \
             tc.tile_pool(name="ps_s", bufs=2, space="PSUM") as ps_s:

            # --- resident constants / weights ---
            xt_sb = res.tile([128, NKC, S], MDT)     # resident x^T
            cos2_sb = res.tile([128, S], MDT)
            sin2_sb = res.tile([128, S], MDT)
            wq_sb = res.tile([128, NKC, W], MDT)
            wk_sb = res.tile([128, NKC, W], MDT)
            wv_sb = res.tile([128, NKC, W], MDT)
            wo_sb = res.tile([128, PAIRS, E], MDT)
            bqc_sb = res.tile([128, PAIRS], F32)
            onesd_sb = res.tile([128, 64], MDT)
            rt_sb = res.tile([128, 128], MDT)
            mask4_sb = res.tile([128, 4, 128], MDT)
            kT_sb = res.tile([128, PAIRS, S], MDT)   # rotated K^T per pair
            v_sb = res.tile([128, S // 128, W], MDT)  # V (seq-major tiles)

            # Startup DMAs: HBM pull runs at a fixed ~260GB/s regardless of
            # queue count, so a single sync queue in strict consumption
            # order is the best schedule. Early bytes are minimized: cos/sin
            # are loaded once for positions 0:1024 (blocks 0-3) and
            # duplicated to the upper partition half SBUF->SBUF on scalar;
            # the 1024:2048 halves ride at the end of the chain.
            # first halves split so pair0's first matmuls can start ~3us
            # earlier while the rest streams in
            nc.sync.dma_start(out=wk_sb[:, 0:4, 0:128], in_=wk[0][:, 0:4, :])
            nc.sync.dma_start(out=xt_sb[:, 0:4, 0:QB], in_=xtc[0][:, 0:4, :])
            nc.sync.dma_start(out=wk_sb[:, 4:8, 0:128], in_=wk[0][:, 4:8, :])
            nc.sync.dma_start(out=xt_sb[:, 4:8, 0:QB], in_=xtc[0][:, 4:8, :])
            nc.sync.dma_start(out=wk_sb[:, :, 128:256], in_=wk[1])
            nc.sync.dma_start(out=wk_sb[:, :, 256:384], in_=wk[2])
            nc.sync.dma_start(out=wk_sb[:, :, 384:512], in_=wk[3])
            nc.sync.dma_start(out=rt_sb, in_=rt)
            nc.sync.dma_start(out=cos2_sb[0:64, 0:1024], in_=cosh[:, 0:1024])
            nc.sync.dma_start(out=sin2_sb[0:64, 0:1024], in_=sinh[:, 0:1024])
            nc.sync.dma_start(out=xt_sb[:, :, QB:2 * QB], in_=xtc[1])
            # dups after xtc1 (their DVE consumers are elastic); same queue
            # as the source loads: DMA->DMA ordering is only guaranteed by
            # queue FIFO
            nc.sync.dma_start(out=cos2_sb[64:128, 0:1024],
                              in_=cos2_sb[0:64, 0:1024])
            nc.sync.dma_start(out=sin2_sb[64:128, 0:1024],
                              in_=sin2_sb[0:64, 0:1024])
            # xtc2 in kc-halves: K(2) can start on the first half while
            # the second is still landing
            nc.sync.dma_start(out=xt_sb[:, 0:4, 2 * QB:3 * QB],
                              in_=xtc[2][:, 0:4, :])
            nc.sync.dma_start(out=xt_sb[:, 4:8, 2 * QB:3 * QB],
                              in_=xtc[2][:, 4:8, :])
            nc.sync.dma_start(out=bqc_sb, in_=bqc)
            for c in range(PAIRS):
                nc.sync.dma_start(out=wq_sb[:, :, c * 128:(c + 1) * 128],
                                  in_=wq[c])
            nc.sync.dma_start(out=xt_sb[:, :, 3 * QB:4 * QB], in_=xtc[3])
            nc.sync.dma_start(out=wv_sb, in_=wv)
            nc.sync.dma_start(out=wo_sb, in_=wo)
            nc.sync.dma_start(out=cos2_sb[0:64, 1024:2048],
                              in_=cosh[:, 1024:2048])
            nc.sync.dma_start(out=sin2_sb[0:64, 1024:2048],
                              in_=sinh[:, 1024:2048])
            nc.sync.dma_start(out=cos2_sb[64:128, 1024:2048],
                              in_=cos2_sb[0:64, 1024:2048])
            nc.sync.dma_start(out=sin2_sb[64:128, 1024:2048],
                              in_=sin2_sb[0:64, 1024:2048])
            # scalar queue: free until its first kraw copy (~13us);
            # DMA->compute deps are tracked across queues, so these can
            # ride here without diluting the sync chain's priority order
            nc.scalar.dma_start(out=mask4_sb, in_=mask4)
            nc.scalar.dma_start(out=onesd_sb, in_=onesd)

            # ---------- Stage A: K^T (rotated) + V for one 256-seq block ----
            a_pend = []
            a_pend_c = []

            def emit_ktail():
                kraw, sl_ = a_pend.pop(0)
                ps_rh = ps.tile([128, QB], F32, tag="dc", bufs=2,
                                name=f"ps_rh_a_{sl_.start}")
                nc.tensor.matmul(ps_rh, rt_sb, kraw, start=True, stop=True)
                rhs_sin = work.tile([128, QB], MDT, tag="rhsin", bufs=3,
                                    name=f"rhs_sin_a_{sl_.start}")
                nc.vector.tensor_mul(out=rhs_sin, in0=ps_rh,
                                     in1=sin2_sb[:, sl_])
                nc.vector.tensor_mul(out=kraw, in0=kraw,
                                     in1=cos2_sb[:, sl_])
                nc.vector.tensor_add(
                    out=kT_sb[:, a_pend_c.pop(0), sl_], in0=kraw,
                    in1=rhs_sin)

            def emit_v(n8):
                sl = slice(n8 * QB, (n8 + 1) * QB)
                xa = xt_sb[:, :, sl]
                for sub in range(2):
                    jb = n8 * 2 + sub
                    ps_v = ps.tile([128, W], F32, tag="pj", bufs=2,
                                   name=f"ps_v_{jb}")
                    for kc in range(NKC):
                        nc.tensor.matmul(
                            ps_v,
                            xa[:, kc, sub * 128:(sub + 1) * 128],
                            wv_sb[:, kc, :],
                            start=(kc == 0), stop=(kc == NKC - 1),
                        )
                    # V copy on DVE: on Scalar it queues ahead of the
                    # exps whose latency gates the sAB score-PSUM ring
                    nc.vector.tensor_copy(out=v_sb[:, jb, :], in_=ps_v)

            def emit_k_pair(n8, c):
                # one K-projection unit: block n8, head pair c
                sl = slice(n8 * QB, (n8 + 1) * QB)
                xa = xt_sb[:, :, sl]
                if c == 0 and 4 <= n8 + 2 <= 7:
                    pre = n8 + 2
                    nc.sync.dma_start(
                        out=xt_sb[:, :, pre * QB:(pre + 1) * QB],
                        in_=xtc[pre])
                ps_k = ps.tile([128, QB], F32, tag="pj", bufs=2,
                               name=f"ps_k_{n8}_{c}")
                for kc in range(NKC):
                    nc.tensor.matmul(
                        ps_k,
                        wk_sb[:, kc, c * 128:(c + 1) * 128],
                        xa[:, kc, :],
                        start=(kc == 0), stop=(kc == NKC - 1),
                    )
                kraw = work.tile([128, QB], MDT, tag="kraw", bufs=4,
                                 name=f"kraw_{n8}_{c}")
                nc.scalar.activation(
                    out=kraw, in_=ps_k,
                    func=mybir.ActivationFunctionType.Copy)
                a_pend.append((kraw, sl))
                a_pend_c.append(c)
                if len(a_pend) > 1:
                    emit_ktail()

            # ---------- Stage B: Q + attention + out-projection -------------
            pending = []          # (qb, c, pABs, presums, cn)
            qb_tails_left = {}
            cn_by_qb = {}
            qrots_by_qb = {}

            def emit_wo(qb):
                cts = cn_by_qb.pop(qb)
                last = qb == NQB - 1
                for sub in range(2):
                    o_sb = work.tile([128, 1024], F32, tag="o_sb",
                                     name=f"o_sb_{qb}_{sub}")
                    rows = slice(qb * QB + sub * 128,
                                 qb * QB + (sub + 1) * 128)
                    for ncol in range(2):
                        # sub1 groups borrow the dc ring so the next
                        # A-burst's first K matmuls (pj ring) don't wait
                        # out-copy latency at the block seam
                        ps_o = ps.tile([128, 512], F32,
                                       tag="pj" if sub == 0 else "dc",
                                       bufs=2,
                                       name=f"ps_o_{qb}_{sub}_{ncol}")
                        for cc in range(PAIRS):
                            nc.tensor.matmul(
                                ps_o,
                                cts[cc][:, sub * 128:(sub + 1) * 128],
                                wo_sb[:, cc, ncol * 512:(ncol + 1) * 512],
                                start=(cc == 0), stop=(cc == PAIRS - 1))
                        osl = slice(ncol * 512, (ncol + 1) * 512)
                        # split copies across Scalar/DVE so exps never queue
                        # behind a burst of out-copies
                        if last:
                            # queues are drained here: halve the copy wall
                            # by splitting across Scalar+DVE
                            nc.scalar.activation(
                                out=o_sb[:, ncol * 512:ncol * 512 + 256],
                                in_=ps_o[:, 0:256],
                                func=mybir.ActivationFunctionType.Copy)
                            nc.vector.tensor_copy(
                                out=o_sb[:, ncol * 512 + 256:(ncol + 1) * 512],
                                in_=ps_o[:, 256:512])
                        elif ncol == 1:
                            nc.vector.tensor_copy(out=o_sb[:, osl],
                                                  in_=ps_o)
                        else:
                            nc.scalar.activation(
                                out=o_sb[:, osl], in_=ps_o,
                                func=mybir.ActivationFunctionType.Copy)
                        if last:
                            eng = (nc.sync, nc.scalar, nc.sync,
                                   nc.gpsimd)[2 * sub + ncol]
                            eng.dma_start(out=out[rows, osl],
                                          in_=o_sb[:, osl])
                    if not last:
                        nc.sync.dma_start(out=out[rows, :], in_=o_sb)

            def emit_tail():
                qb, c, pABs, presums, cn = pending.pop(0)
                ps_dcv = ps.tile([128, 2, QB], F32, tag="dc", bufs=2,
                                 name=f"ps_dcv_{qb}_{c}")
                # Denominators: ones-matmul over the presummed planes.
                # Side A -> partitions 0:64, side B -> 64:128 (tile_position).
                # All den/PV matmuls are emitted A/B-interleaved: the sides
                # occupy disjoint 64-col groups, so adjacent MMs overlap.
                def tp_hs(side):
                    return ((None, (0, 64))[side],
                            slice(64 * side, 64 * side + 64))
                if qb == 0:
                    for mmi in range(2):
                        for side in range(2):
                            tp, hs = tp_hs(side)
                            if mmi == 0:
                                # h0 den = masked diag plane only
                                nc.tensor.matmul(
                                    ps_dcv[hs, 0, 0:128], onesd_sb,
                                    pABs[side][:, 4, :], start=True,
                                    stop=True, tile_position=tp)
                            else:
                                nc.tensor.matmul(
                                    ps_dcv[hs, 0, 128:256], onesd_sb,
                                    presums[side][:, 1, :], start=True,
                                    stop=True, tile_position=tp)
                elif qb == NQB - 1:
                    # Last block: no presum (GpSimd chain would sit on
                    # the drain critical path) - accumulate planes on PE.
                    for half in range(2):
                        csl = slice(half * 128, (half + 1) * 128)
                        for i, m in enumerate((half, half + 2, half + 4)):
                            for side in range(2):
                                tp, hs = tp_hs(side)
                                nc.tensor.matmul(
                                    ps_dcv[hs, 0, csl], onesd_sb,
                                    pABs[side][:, m, :], start=(i == 0),
                                    stop=(i == 2), tile_position=tp)
                else:
                    for side in range(2):
                        tp, hs = tp_hs(side)
                        nc.tensor.matmul(
                            ps_dcv[hs, 0, :], onesd_sb,
                            presums[side][:, 0:2, :], start=True, stop=True,
                            tile_position=tp)
                # PV: accumulate banded key blocks into C^T [dims, 256].
                # Full-width (N=256) matmuls first so the accumulation group
                # opens with every byte written (PSUM zero-region rule).
                if qb == 0:
                    for mmi in range(2):
                        for side in range(2):
                            tp, hs = tp_hs(side)
                            pAB = pABs[side]
                            vcol = c * 128 + 64 * side
                            if mmi == 0:
                                nc.tensor.matmul(
                                    ps_dcv[hs, 1, :],
                                    v_sb[:, 0, vcol:vcol + 64],
                                    pAB[:, 4:6, :], start=True, stop=False,
                                    tile_position=tp)
                            else:
                                nc.tensor.matmul(
                                    ps_dcv[hs, 1, 128:256],
                                    v_sb[:, 1, vcol:vcol + 64],
                                    pAB[:, 1, :], start=False, stop=True,
                                    tile_position=tp)
                else:
                    jb = 2 * qb
                    for mmi in range(4):
                        for side in range(2):
                            tp, hs = tp_hs(side)
                            pAB = pABs[side]
                            vcol = c * 128 + 64 * side
                            if mmi == 0:
                                nc.tensor.matmul(
                                    ps_dcv[hs, 1, :],
                                    v_sb[:, jb - 1, vcol:vcol + 64],
                                    pAB[:, 2:4, :], start=True, stop=False,
                                    tile_position=tp)
                            elif mmi == 1:
                                nc.tensor.matmul(
                                    ps_dcv[hs, 1, :],
                                    v_sb[:, jb, vcol:vcol + 64],
                                    pAB[:, 4:6, :], start=False, stop=False,
                                    tile_position=tp)
                            elif mmi == 2:
                                nc.tensor.matmul(
                                    ps_dcv[hs, 1, 0:128],
                                    v_sb[:, jb - 2, vcol:vcol + 64],
                                    pAB[:, 0, :], start=False, stop=False,
                                    tile_position=tp)
                            else:
                                nc.tensor.matmul(
                                    ps_dcv[hs, 1, 128:256],
                                    v_sb[:, jb + 1, vcol:vcol + 64],
                                    pAB[:, 1, :], start=False, stop=True,
                                    tile_position=tp)
                recip = work.tile([128, QB], F32, tag="recip", bufs=2,
                                  name=f"recip_{qb}_{c}")
                nc.vector.reciprocal_approx_fast(out=recip,
                                                 in_=ps_dcv[:, 0, :])
                nc.vector.tensor_mul(out=cn, in0=ps_dcv[:, 1, :], in1=recip)
                qb_tails_left[qb] -= 1
                if qb_tails_left[qb] == 0:
                    emit_wo(qb)

            def emit_q(qb, c, qrots):
                qsl = slice(qb * QB, (qb + 1) * QB)
                xq = xt_sb[:, :, qsl]
                ps_q = ps.tile([128, QB], F32, tag="pj", bufs=2,
                               name=f"ps_q_{qb}_{c}")
                for kc in range(NKC):
                    nc.tensor.matmul(
                        ps_q,
                        wq_sb[:, kc, c * 128:(c + 1) * 128],
                        xq[:, kc, :],
                        start=(kc == 0), stop=(kc == NKC - 1),
                    )
                qraw = work.tile([128, QB], MDT, tag="kraw", bufs=4,
                                 name=f"qraw_{qb}_{c}")
                nc.scalar.activation(
                    out=qraw, in_=ps_q,
                    func=mybir.ActivationFunctionType.Identity,
                    bias=bqc_sb[:, c:c + 1])
                ps_rh = ps.tile([128, QB], F32, tag="dc", bufs=2,
                                name=f"ps_rh_{qb}_{c}")
                nc.tensor.matmul(ps_rh, rt_sb, qraw, start=True,
                                 stop=True)
                qrot = work.tile([128, QB], MDT, tag="qrot",
                                 name=f"qrot_{qb}_{c}", bufs=5)
                rhs_sin = work.tile([128, QB], MDT, tag="rhsin", bufs=3,
                                    name=f"rhs_sin_{qb}_{c}")
                nc.vector.tensor_mul(out=rhs_sin, in0=ps_rh,
                                     in1=sin2_sb[:, qsl])
                nc.vector.tensor_mul(out=qraw, in0=qraw,
                                     in1=cos2_sb[:, qsl])
                nc.vector.tensor_add(out=qrot, in0=qraw, in1=rhs_sin)
                qrots.append(qrot)

            def emit_attn(qb, c, qrots):
                cn = work.tile([128, QB], MDT, tag="cn", bufs=10,
                               name=f"cn_{qb}_{c}")
                cn_by_qb[qb].append(cn)
                jb = 2 * qb
                pABs = []
                presums = []
                q2 = qrots[c]
                sABs = []
                for side in range(2):
                    sABs.append(ps_s.tile([128, 6, 128], F32, tag="sAB",
                                          bufs=2,
                                          name=f"sAB_{qb}_{c}_{side}"))
                # Score matmuls interleaved A/B: the two sides contract over
                # disjoint 64-row groups, so adjacent MMs run concurrently.
                if qb == 0:
                    for mmi in range(2):
                        for side in range(2):
                            hs = slice(64 * side, 64 * side + 64)
                            if mmi == 0:
                                nc.tensor.matmul(
                                    sABs[side][:, 4:6, :],
                                    kT_sb[hs, c, 0:128],
                                    q2[hs, :], start=True, stop=True)
                            else:
                                nc.tensor.matmul(
                                    sABs[side][:, 1, :],
                                    kT_sb[hs, c, 128:256],
                                    q2[hs, 128:256], start=True, stop=True)
                else:
                    for mmi in range(4):
                        for side in range(2):
                            hs = slice(64 * side, 64 * side + 64)
                            sAB = sABs[side]
                            if mmi == 0:
                                nc.tensor.matmul(
                                    sAB[:, 2:4, :],
                                    kT_sb[hs, c, (jb - 1) * 128:jb * 128],
                                    q2[hs, :], start=True, stop=True)
                            elif mmi == 1:
                                nc.tensor.matmul(
                                    sAB[:, 4:6, :],
                                    kT_sb[hs, c, jb * 128:(jb + 1) * 128],
                                    q2[hs, :], start=True, stop=True)
                            elif mmi == 2:
                                nc.tensor.matmul(
                                    sAB[:, 0, :],
                                    kT_sb[hs, c, (jb - 2) * 128:(jb - 1) * 128],
                                    q2[hs, 0:128], start=True, stop=True)
                            else:
                                nc.tensor.matmul(
                                    sAB[:, 1, :],
                                    kT_sb[hs, c, (jb + 1) * 128:(jb + 2) * 128],
                                    q2[hs, 128:256], start=True, stop=True)
                for side in range(2):
                    sAB = sABs[side]
                    pAB = attn.tile([128, 6, 128], MDT, tag="pAB",
                                    bufs=12,
                                    name=f"pAB_{qb}_{c}_{side}")
                    if qb == 0:
                        nc.scalar.activation(
                            out=pAB[:, 4:6, :], in_=sAB[:, 4:6, :],
                            func=mybir.ActivationFunctionType.Exp,
                            scale=SCALE)
                        nc.scalar.activation(
                            out=pAB[:, 1, :], in_=sAB[:, 1, :],
                            func=mybir.ActivationFunctionType.Exp,
                            scale=SCALE)
                    else:
                        nc.scalar.activation(
                            out=pAB[:, 0:6, :], in_=sAB[:, 0:6, :],
                            func=mybir.ActivationFunctionType.Exp,
                            scale=SCALE)
                    # masks: left planes (0,3) and diag planes (1,4)
                    if qb > 0:
                        pv4 = pAB[:, 0:6, :].rearrange(
                            "p (a b) f -> p a b f", b=3)[:, :, 0:2, :]
                        nc.vector.tensor_mul(
                            out=pv4, in0=pv4,
                            in1=mask4_sb[:, 0:4, :].rearrange(
                                "p (a b) f -> p a b f", b=2))
                    else:
                        nc.vector.tensor_mul(
                            out=pAB[:, 1:5:3, :],
                            in0=pAB[:, 1:5:3, :], in1=mask4_sb[:, 1:4:2, :])
                    # presum the 3 key-block planes (GpSimd)
                    presum = work.tile([128, 2, 128], MDT, tag="psum",
                                       bufs=12,
                                       name=f"presum_{qb}_{c}_{side}")
                    if qb == 0:
                        nc.gpsimd.tensor_add(
                            out=presum[:, 1, :], in0=pAB[:, 1, :],
                            in1=pAB[:, 5, :])
                    elif qb < NQB - 1:
                        nc.gpsimd.tensor_add(
                            out=presum, in0=pAB[:, 0:2, :],
                            in1=pAB[:, 2:4, :])
                        nc.gpsimd.tensor_add(
                            out=presum, in0=presum, in1=pAB[:, 4:6, :])
                    pABs.append(pAB)
                    presums.append(presum)
                pending.append((qb, c, pABs, presums, cn))
                # qb0 tails need v_sb[0:2], which V(0) only writes during
                # the qb=1 slot - hold all qb0 tails until then
                if len(pending) > DEPTH and qb > 0:
                    emit_tail()

            # ---------- interleaved schedule --------------------------------
            # A(n8) = K(n8) + Q(n8-2) + V(n8-1); B(qb) right after its Q.
            # The bursty A-phase between attention bursts gives the PE a
            # ~4.7us buffer that hides the exp/cn chains on Scalar/DVE.
            for c in range(PAIRS):
                emit_k_pair(0, c)
            for c in range(PAIRS):
                emit_k_pair(1, c)
            for qb in range(NQB):
                if qb + 2 <= NQB - 1:
                    n8 = qb + 2
                    for c in range(PAIRS):
                        # defer K(7)'s last two pairs into the thin qb=6
                        # slot so the drain keeps PE buffer work
                        if n8 < NQB - 1 or c < 2:
                            emit_k_pair(n8, c)
                        emit_q(qb, c, qrots_by_qb.setdefault(qb, []))
                    if n8 == 3:
                        # V(0..2) here so wv can load late in the startup
                        # window without stalling the PE
                        emit_v(0)
                        emit_v(1)
                        emit_v(2)
                    elif n8 >= 4:
                        emit_v(n8 - 1)
                else:
                    # mini-slots: Q6 (+ deferred K7 pairs + V7 + K-rot
                    # flush) and Q7
                    for c in range(PAIRS):
                        if qb == NQB - 2 and c >= 2:
                            emit_k_pair(NQB - 1, c)
                        emit_q(qb, c, qrots_by_qb.setdefault(qb, []))
                    if qb == NQB - 2:
                        while a_pend:
                            emit_ktail()
                        emit_v(NQB - 1)
                qb_tails_left[qb] = PAIRS
                cn_by_qb[qb] = []
                qrots = qrots_by_qb[qb]
                for c in range(PAIRS):
                    emit_attn(qb, c, qrots)
                    if qb == NQB - 1:
                        # drain the tail pipeline gradually so the DVE
                        # recip/cn chain keeps up and the final
                        # out-projection isn't serialized behind 3 tails
                        while len(pending) > max(1, DEPTH - 1 - c):
                            emit_tail()
            while pending:
                emit_tail()

    nc.compile()
    return nc


def _host_consts():
    R64 = np.zeros((64, 64), np.float32)
    for d in range(32):
        R64[d, d + 32] = -1.0
    for d in range(32, 64):
        R64[d, d - 32] = 1.0
    Rblk = np.zeros((128, 128), np.float32)
    Rblk[:64, :64] = R64
    Rblk[64:, 64:] = R64
    rt = np.ascontiguousarray(Rblk.T)

    pv, fv = np.meshgrid(np.arange(128), np.arange(128), indexing="ij")
    maskl = (fv <= pv).astype(np.float32)   # left block: valid q <= k
    maskd = (fv >= pv).astype(np.float32)   # diag block: valid k <= q
    mask4 = np.stack([maskl, maskd, maskl, maskd], axis=1)
    return rt, mask4


def _make_in_maps(x, cos, sin, Wq, bq, Wk, Wv, Wo):
    import ml_dtypes
    mdt_np = np.dtype(ml_dtypes.bfloat16)
    rt, mask4 = _host_consts()
    onesd = np.ones((128, 64), mdt_np)
    in_maps = []
    for core in range(8):
        b, g = core // 2, core % 2
        gs = slice(g * W, (g + 1) * W)
        cosT = np.ascontiguousarray(cos[b].T)
        sinT = np.ascontiguousarray(sin[b].T)
        xT = x[b].T.astype(mdt_np)  # [1024, 2048]
        xtc = np.ascontiguousarray(
            xT.reshape(8, 128, 8, 256).transpose(2, 1, 0, 3))

        def wchunks(Wm):
            # [4, 128, NKC, 128]: per head-pair contiguous DMA chunks
            wfull = Wm[:, gs].reshape(8, 128, 512).transpose(1, 0, 2)
            return np.ascontiguousarray(
                wfull.reshape(128, 8, 4, 128).transpose(2, 0, 1, 3)
            ).astype(mdt_np)

        in_maps.append({
            "xtc": xtc,
            "wq": wchunks(Wq),
            "wk": wchunks(Wk),
            "wv": np.ascontiguousarray(
                Wv[:, gs].reshape(8, 128, 512).transpose(1, 0, 2)
            ).astype(mdt_np),
            "wo": np.ascontiguousarray(
                Wo[gs, :].reshape(4, 128, 1024).transpose(1, 0, 2)
            ).astype(mdt_np),
            "bqc": np.ascontiguousarray(
                bq[gs].reshape(PAIRS, 128).T).astype(np.float32),
            "onesd": onesd,
            "rt": rt.astype(mdt_np),
            "cosh": cosT.astype(mdt_np),
            "sinh": sinT.astype(mdt_np),
            "mask4": mask4.astype(mdt_np),
        })
    return in_maps


def _get_nc():
    global _STATE
    if _STATE is None:
        _STATE = _build()
    return _STATE


def run(inputs, trace=False, trace_cores=None):
    """Run the SPMD kernel; returns (full_output, BassKernelResults)."""
    nc = _get_nc()
    in_maps = _make_in_maps(
        inputs["x"], inputs["cos"], inputs["sin"], inputs["Wq"], inputs["bq"],
        inputs["Wk"], inputs["Wv"], inputs["Wo"])
    res = bass_utils.run_bass_kernel_spmd(
        nc, in_maps, core_ids=list(range(8)), trace=trace,
        trace_cores=trace_cores)
    mask = np.asarray(inputs["mask"])
    bo = np.asarray(inputs["bo"])
    bv = np.asarray(inputs["bv"])
    Wo = np.asarray(inputs["Wo"])
    out = np.zeros((B, S, E), np.float32)
    for core in range(8):
        b = core // 2
        out[b] += res.results[core]["out"]
    out += (bv @ Wo + bo)[None, None, :]
    out *= mask[..., None].astype(np.float32)
    return out, res


def kernel(**inputs) -> np.ndarray:
    inputs = {k: np.asarray(v) for k, v in inputs.items()}
    out, _ = run(inputs)
    return out

